# revision 32
# baseline (speedup 1.0000x reference)
"""v9: single u-projection, raw-x score stationaries, packed 58-col queries.

Softmax is invariant to per-query constants, so with
  u = (Wq65 @ Wk^T)^T @ x65          ([64, pix], one projection)
the scores  s'[kpix, q] = sum_r u[r, q] * x[r, kpix]  equal the true
q.k up to a per-query constant (the bk cross term) that cancels in the
softmax. q and k are never materialized: the score stationary is RAW x,
which is loaded duplicated into both partition halves straight from
DRAM (no SBUF->SBUF duplication), and only u needs a [Mu||Mu] stacked
projection.

Queries are streamed pre-sliced to the 58 valid window columns
(u[:, rows, 3:61]) so score tiles are [128, 8, 116] — no padded
columns, no pad-handling mask row; output partitions 0:116 map 1:1 to
window columns.

2-pair score tiles as v8: all 8 slots used, 5 score matmuls per tile
(absolute-chunk-parity row groups, shared chunks stream both pairs'
queries at once), exp/mask-mul once per tile, one PSUM->SBUF cast per
tile. Normalization on the host (exp carries bias -4 for f16 range).

Slot layout per tile (pairs p=2t, p2=p+1; chunk s = key rows 2s,2s+1):
  lo (x rows in partitions 0:64, even chunks):   slot 0 = (p, c0),
      slots 1,2 = chunk p+2 -> (p, c2), (p2, c1), slot 3 = (p2, c3)
  hi (odd chunks): slots 4,5 = chunk p+1 -> (p, c1), (p2, c0)
      slots 6,7 = chunk p+3 -> (p, c3), (p2, c2)
c_of_slot = [0, 2, 1, 3, 1, 0, 3, 2].
"""

import numpy as np

B, C, H, W, K = 8, 64, 64, 64, 7
HC = WC = H - K + 1          # 58
N = HC * WC                  # 3364
NPAIR = HC // 2              # 29 window-row pairs
NTILE = NPAIR // 2           # 14 full 2-pair tiles (+1 single-pair tail)
JW = 2 * WC                  # 116 query columns per tile-slot
SCALE = float(1.0 / np.sqrt(C))
C_OF_SLOT = [0, 2, 1, 3, 1, 0, 3, 2]
SLOT_OF_PAIR_CHUNK = ([0, 4, 1, 6], [5, 2, 7, 3])   # [pair-in-tile][chunk]

_CACHE = {}


def _build_mask_np():
    """[128, 8, 116] band mask, slot order C_OF_SLOT, packed query cols."""
    kk = np.arange(128)[:, None, None]
    c = np.arange(4)[None, :, None]
    col = np.arange(JW)[None, None, :]
    k_local = c * 128 + kk
    dI, jp = k_local // W, k_local % W
    jb, j = col // WC, col % WC
    ok = (dI - jb >= 0) & (dI - jb < K) & (jp - j >= 0) & (jp - j < K)
    m = ok.astype(np.float16)
    return np.ascontiguousarray(m[:, C_OF_SLOT, :])    # [128, 8, 116]


def _build_module():
    import concourse.tile as tile
    from concourse import bacc, mybir

    dt = mybir.dt
    f32 = dt.float32
    f16 = dt.float16

    nc = bacc.Bacc(
        "TRN2", target_bir_lowering=False, debug=False, enable_asserts=False,
        num_devices=8,
    )

    x_d = nc.dram_tensor("x65", [65, H, W], f16, kind="ExternalInput").ap()
    # weights: [128, 192] f16 = wuu[65->128, 128] ++ wv[65->128, 64]
    w_d = nc.dram_tensor("weights", [128, 192], f16, kind="ExternalInput").ap()
    # mask: [128, 2, 464] f16 (2 row-groups x 4 packed 116-col slots)
    mk_d = nc.dram_tensor("mask", [128, 2 * 4 * JW], f16, kind="ExternalInput").ap()
    # out = unnormalized numerator (0:64) ++ softmax denominator (64)
    out_d = nc.dram_tensor("out", [128, NPAIR, C + 1], f16, kind="ExternalOutput").ap()

    with tile.TileContext(nc) as tc:
        with (
            tc.tile_pool(name="const", bufs=1) as const,
            tc.tile_pool(name="qk", bufs=1) as qkpool,
            tc.tile_pool(name="attn", bufs=4) as attnpool,
        ):
            x_sb = const.tile([65, H, W], f16)
            # raw x split by 2-row chunk parity: even chunks (rows 4m,4m+1)
            # in partitions 0:64, odd chunks in 64:128
            xd_sb = const.tile([128, 16, 2, W], f16)
            w_sb = const.tile([128, 192], f16)
            mk_sb = const.tile([128, 2, 4 * JW], f16)
            stage_sb = const.tile([128, NPAIR, C + 1], f16)
            nbias_sb = const.tile([128, 1], f32)
            # u pre-packed to the 58 valid window cols so score streams are
            # contiguous; duplicated in both partition halves
            u_sb = qkpool.tile([128, H, WC], f16, tag="u")
            v_sb = qkpool.tile([128, 32, C + 1], f16, tag="v")

            wuu_sb = w_sb[0:65, 0:128]         # [Mu || Mu]
            wv_sb = w_sb[0:65, 128:192]
            mask_sb = mk_sb[:]

            # inputs: tiny weights DMA first so projections start early;
            # x65 in 4 chunks across queues; the score stationaries load
            # straight from DRAM with chunk-parity partition split
            # all input DMA dispatch stays off the Scalar engine so the
            # u/v psum copies (and later the exps) never queue behind the
            # ~1us DMA dispatch cost
            xq = x_d[0:64, :, :].rearrange("c (m q) w -> c m q w", q=4)
            nc.sync.dma_start(w_sb[:], w_d[:])
            nc.sync.dma_start(x_sb[:, 0:16, :], x_d[:, 0:16, :])
            nc.scalar.dma_start(x_sb[:, 16:32, :], x_d[:, 16:32, :])
            nc.sync.dma_start(x_sb[:, 32:48, :], x_d[:, 32:48, :])
            nc.scalar.dma_start(x_sb[:, 48:64, :], x_d[:, 48:64, :])
            nc.gpsimd.dma_start(xd_sb[0:64, :, :, :], xq[:, :, 0:2, :])
            nc.gpsimd.dma_start(xd_sb[64:128, :, :, :], xq[:, :, 2:4, :])
            nc.sync.dma_start(mk_sb[:], mk_d[:])
            nc.gpsimd.memset(v_sb[:, :, C:C + 1], 1.0)
            nc.gpsimd.memset(nbias_sb[:], -4.0)

            # ---- u and v projections ----
            with (
                tc.tile_pool(name="psu", bufs=2, space="PSUM") as psu,
                tc.tile_pool(name="psv", bufs=2, space="PSUM") as psv,
            ):
                for g in range(4):
                    ps = psu.tile([128, 16, W], f32, tag="ps")
                    for h in range(2):
                        s = 2 * g + h
                        nc.tensor.matmul(
                            ps[:, 8 * h:8 * h + 8, :],
                            wuu_sb,
                            x_sb[:, s * 8:(s + 1) * 8, :],
                        )
                    eng = nc.scalar.copy if g % 2 == 0 else nc.vector.tensor_copy
                    eng(u_sb[:, g * 16:(g + 1) * 16, :], ps[:, :, 3:3 + WC])
                for g in range(8):
                    ps = psv.tile([128, 4, C], f32, tag="psv")
                    for h in range(4):
                        r = 4 * g + h
                        nc.tensor.matmul(
                            ps[:, h, :],
                            x_sb[:, 2 * r:2 * r + 2, :],
                            wv_sb,
                        )
                    eng = nc.scalar.copy if g % 2 == 1 else nc.vector.tensor_copy
                    eng(v_sb[:, 4 * g:4 * g + 4, 0:C], ps[:])

            # ---- banded attention, 2 pairs per tile ----
            with (
                tc.tile_pool(name="pssc", bufs=3, space="PSUM") as pssc,
                tc.tile_pool(name="psout", bufs=2, space="PSUM") as psout,
            ):
                scores = [None] * (NTILE + 1)

                def xch(s, half):
                    base = 64 * half
                    return xd_sb[base:base + 64, s // 2, :, :]

                def ust(half, r0, r1):
                    base = 64 * half
                    return u_sb[base:base + 64, r0:r1, :]

                def emit_scores(t):
                    p = 2 * t
                    i = 2 * p              # first key row of chunk p
                    # [128, 2, 512]: one PSUM bank per row group; 4 packed
                    # 116-col slots per group (464 used, 48 pad)
                    sc = pssc.tile([128, 2, 512], f32, tag="sc")
                    if t < NTILE:
                        # lo: even chunks p, p+2, p+4 -> slots 0 | 1,2 | 3
                        nc.tensor.matmul(sc[:, 0, 0:JW], xch(p, 0),
                                         ust(0, i + 3, i + 5))
                        nc.tensor.matmul(sc[:, 0, JW:3 * JW], xch(p + 2, 0),
                                         ust(0, i + 3, i + 7))
                        nc.tensor.matmul(sc[:, 0, 3 * JW:4 * JW], xch(p + 4, 0),
                                         ust(0, i + 5, i + 7))
                        # hi: odd chunks p+1, p+3 -> slots 4,5 | 6,7
                        nc.tensor.matmul(sc[:, 1, 0:2 * JW], xch(p + 1, 1),
                                         ust(1, i + 3, i + 7))
                        nc.tensor.matmul(sc[:, 1, 2 * JW:4 * JW], xch(p + 3, 1),
                                         ust(1, i + 3, i + 7))
                    else:
                        # last single pair 28: chunks 28,30 -> lo slots 0,1;
                        # 29,31 -> hi slots 4,5 (mask for slot5 uses c3)
                        nc.tensor.matmul(sc[:, 0, 0:JW], xch(p, 0),
                                         ust(0, i + 3, i + 5))
                        nc.tensor.matmul(sc[:, 0, JW:2 * JW], xch(p + 2, 0),
                                         ust(0, i + 3, i + 5))
                        nc.tensor.matmul(sc[:, 1, 0:JW], xch(p + 1, 1),
                                         ust(1, i + 3, i + 5))
                        nc.tensor.matmul(sc[:, 1, JW:2 * JW], xch(p + 3, 1),
                                         ust(1, i + 3, i + 5))
                    scores[t] = sc

                def slot_ap(tile_ap, sl):
                    # full 128-wide slot: cols 116:128 are garbage that only
                    # reaches unused output partitions 116:128, and the
                    # aligned width keeps LDWEIGHTS on the fast path
                    g, idx = sl // 4, sl % 4
                    return tile_ap[:, g, idx, :]

                def emit_tail(t):
                    sc = scores[t]
                    p = 2 * t
                    # ex/at slots padded to 128 cols so AV weight loads hit
                    # the 32-aligned fast path (cols 116:128 stay garbage and
                    # only reach the unused output partitions 116:128)
                    ex = attnpool.tile([128, 2, 4, 128], f16, tag="ex")
                    at = attnpool.tile([128, 2, 4, 128], f16, tag="at")
                    nc.gpsimd.memset(at[:, :, :, JW:128], 0.0)
                    npair_t = 2 if t < NTILE else 1
                    if t < NTILE:
                        sc4 = sc[:, :, 0:4 * JW].rearrange(
                            "p g (i j) -> p g i j", i=4)
                        nc.scalar.activation(
                            ex[:, :, :, 0:JW], sc4,
                            mybir.ActivationFunctionType.Exp,
                            scale=SCALE, bias=nbias_sb[:],
                        )
                        mk4 = mask_sb.rearrange("p g (i j) -> p g i j", i=4)
                        nc.vector.tensor_mul(at[:, :, :, 0:JW],
                                             ex[:, :, :, 0:JW], mk4)
                        slots_of_pair = SLOT_OF_PAIR_CHUNK
                    else:
                        sc4 = sc[:, :, 0:4 * JW].rearrange(
                            "p g (i j) -> p g i j", i=4)
                        mk4 = mask_sb.rearrange("p g (i j) -> p g i j", i=4)
                        nc.scalar.activation(
                            ex[:, :, 0:2, 0:JW], sc4[:, :, 0:2, :],
                            mybir.ActivationFunctionType.Exp,
                            scale=SCALE, bias=nbias_sb[:],
                        )
                        nc.vector.tensor_mul(at[:, 0, 0:2, 0:JW],
                                             ex[:, 0, 0:2, 0:JW],
                                             mk4[:, 0, 0:2, 0:JW])
                        nc.vector.tensor_mul(at[:, 1, 0, 0:JW],
                                             ex[:, 1, 0, 0:JW],
                                             mk4[:, 1, 0, 0:JW])
                        nc.vector.tensor_mul(at[:, 1, 1, 0:JW],
                                             ex[:, 1, 1, 0:JW],
                                             mk4[:, 1, 2, 0:JW])
                        slots_of_pair = ([0, 4, 1, 5],)
                    ops = psout.tile([128, 2, C + 1], f32, tag="ops")
                    for pp in range(npair_t):
                        slots = slots_of_pair[pp]
                        for c in range(4):
                            nc.tensor.matmul(
                                ops[:, pp, :],
                                slot_ap(at, slots[c]),
                                v_sb[:, p + pp + c, :],
                                start=(c == 0), stop=(c == 3),
                            )
                    nc.vector.tensor_copy(
                        stage_sb[0:JW, p:p + npair_t, :],
                        ops[0:JW, 0:npair_t, :],
                    )
                    flush_i0 = {1: 0, 3: 4, 5: 8, 7: 12, 9: 16, 11: 20,
                                13: 24, 14: 28}
                    if t in flush_i0:
                        i0 = flush_i0[t]
                        p1 = p + npair_t
                        eng = nc.sync if t in (3, 7, 11, 14) else nc.gpsimd
                        eng.dma_start(
                            out_d[0:JW, i0:p1, :], stage_sb[0:JW, i0:p1, :],
                        )

                LAGT = 2
                for t in range(NTILE + 1):
                    emit_scores(t)
                    if t >= LAGT:
                        emit_tail(t - LAGT)
                for t in range(NTILE + 1 - LAGT, NTILE + 1):
                    emit_tail(t)

    nc.compile()
    return nc


def _get_module():
    if "nc" not in _CACHE:
        _CACHE["nc"] = _build_module()
        _CACHE["mask"] = _build_mask_np()
    return _CACHE["nc"], _CACHE["mask"]


def _make_in_maps(x, Wq, bq, Wk, bk, Wv, bv, mask):
    wq65 = np.concatenate([Wq, bq[None]]).astype(np.float32)
    mu = (wq65 @ Wk.T.astype(np.float32)).astype(np.float16)   # [65, 64]
    wv65 = np.concatenate([Wv, bv[None]]).astype(np.float16)
    wuu = np.zeros((128, 128), np.float16)
    wuu[0:65, 0:64] = mu
    wuu[0:65, 64:128] = mu
    wv = np.zeros((128, 64), np.float16)
    wv[0:65] = wv65
    weights = np.ascontiguousarray(np.concatenate([wuu, wv], axis=1))
    maskf = np.ascontiguousarray(mask.reshape(128, 8 * JW))
    ones = np.ones((1, H, W), np.float16)
    in_maps = []
    for b in range(B):
        x65 = np.concatenate([np.asarray(x[b]).astype(np.float16), ones])
        in_maps.append({
            "x65": np.ascontiguousarray(x65),
            "weights": weights,
            "mask": maskf,
        })
    return in_maps


def _unstage(arr):
    """[128, NPAIR, C+1] f16 num/den staging -> [HC, WC, C] f32."""
    a = arr.astype(np.float32)
    lo = a[0:WC]             # window rows 2i
    hi = a[WC:2 * WC]        # window rows 2i+1
    out = np.empty((HC, WC, C), np.float32)
    out[0::2] = (lo[:, :, 0:C] / lo[:, :, C:C + 1]).transpose(1, 0, 2)
    out[1::2] = (hi[:, :, 0:C] / hi[:, :, C:C + 1]).transpose(1, 0, 2)
    return out


def run(inputs, trace=False, **spmd_kwargs):
    from concourse import bass_utils

    nc, mask = _get_module()
    in_maps = _make_in_maps(
        inputs["x"], inputs["Wq"], inputs["bq"], inputs["Wk"], inputs["bk"],
        inputs["Wv"], inputs["bv"], mask,
    )
    res = bass_utils.run_bass_kernel_spmd(
        nc, in_maps, core_ids=list(range(B)), trace=trace, **spmd_kwargs,
    )
    out = np.stack([_unstage(res.results[b]["out"]) for b in range(B)])
    return out, res


def kernel(**inputs) -> np.ndarray:
    return run(inputs)[0]
